# revision 1
# baseline (speedup 1.0000x reference)
"""nn_GCNConv Trainium2 Bass kernel (8 NeuronCores, SPMD, no collectives).

Computation: out = segment_sum(features[src], dst, N) @ W + b
  features [10000,128] f32, edge_index [2,640000] i64, W [128,256], b [256]

Sharding strategy (dst-node sharding -> no cross-core reduce needed):
  - 10240 node slots = 80 windows of 128 nodes; core c owns windows
    10c..10c+9 (nodes [1280c, 1280c+1280)).
  - The host groups edges by destination window (this is the edge shard),
    pads each window's edge list to a uniform number of 128-edge chunks
    (pad: src index 0 with local-dst sentinel -1 -> contributes zero).
  - Per core on device, per window:
      * dma_gather     G[e,:]  = feat_bf16[src[e],:]      (SWDGE row gather)
      * DVE is_equal   H[e,j]  = (local_dst[e] == j)      (one-hot, bf16)
      * PE             aggT   += G_chunk^T @ H_chunk       (PSUM f32 accum)
      * PE             out     = aggT^T @ W ; DVE adds b   (f32)
  - Host concatenates the 8 per-core [1280,256] outputs and truncates to
    10000 rows. Only slicing/packing happens on host; all arithmetic on
    feature values runs on device.
"""

import sys

import numpy as np

_TRN_REPO = "/opt/trn_rl_repo"
if _TRN_REPO not in sys.path:
    sys.path.insert(0, _TRN_REPO)

import concourse.bass as bass  # noqa: E402
import concourse.mybir as mybir  # noqa: E402
import concourse.tile as tile  # noqa: E402
from concourse import bacc, bass_utils  # noqa: E402

# ---------------------------------------------------------------------------
# Workaround: this walrus build rejects >1 sync-wait on a CTRL instruction
# ("Too many sync wait commands"). Tile's tail drain attaches a wait for every
# live sem lane to one InstDrain; chunk them across single-wait nops instead.
import re as _re  # noqa: E402

import bass_rust as _bass_rust  # noqa: E402


def _clock_ticks(vc):
    m = _re.search(r"\[([0-9, ]*)\]", repr(vc))
    return [int(x) for x in m.group(1).split(",")] if m.group(1).strip() else []


def _drain_and_barrier(self, tick_clock, wait_clock):
    ticks = _clock_ticks(tick_clock.global_clock)
    nz = [(i, t) for i, t in enumerate(ticks) if t > 0]
    for i, t in nz:
        vc = _bass_rust.VectorClock()
        vc.require_at_least(i, t)
        nop = self.nc.sync.nop(nofuse=True, hint="tail_wait")
        wait_clock.add_sem_waits(nop.ins, tile.ScopedClock({None: vc}))
    self.nc.sync.drain()  # waits already carried by the nops (SP FIFO order)
    self.nc.all_engine_barrier()
    assert self.sems is not None
    popped = self.nc._tile_sem_poison_stack.pop()
    assert popped is self._sem_poison
    self.nc.clear_and_free_semaphores(list(self.sems.allocated().values()))
    self.nc.all_engine_barrier()


tile.TileContext._drain_and_barrier = _drain_and_barrier
# ---------------------------------------------------------------------------

P = 128            # SBUF partitions = window node count = edge chunk size
C_IN = 128
C_OUT = 256
N_NODES = 10000
N_CORES = 8
WPC = 10           # windows per core
MODE = "bf16_pre"  # "f32" | "bf16_pre" | "bf16_dve"
GATHER_GROUP = 8   # 128-idx chunks per dma_gather call (SWDGE ring limit)


def _build_kernel(n_feat_rows: int, wpc: int, nch: int, mode: str):
    idxcols = nch * P // 16
    nc = bacc.Bacc("TRN2", num_swdge_queues=4, dynamic_dma_scratch_size=65536)
    dt = mybir.dt
    mm_dt = dt.float32 if mode == "f32" else dt.bfloat16

    feat = nc.dram_tensor("feat", [n_feat_rows, C_IN], dt.float32, kind="ExternalInput")
    w_d = nc.dram_tensor("w", [C_IN, C_OUT], dt.float32, kind="ExternalInput")
    bb_d = nc.dram_tensor("bb", [P, C_OUT], dt.float32, kind="ExternalInput")
    iota_d = nc.dram_tensor("iota", [P, P], dt.float32, kind="ExternalInput")
    idxs_d = nc.dram_tensor("idxs", [P, wpc * idxcols], dt.int16, kind="ExternalInput")
    dstloc_d = nc.dram_tensor("dstloc", [P, wpc * nch], dt.float32, kind="ExternalInput")
    out_d = nc.dram_tensor("out", [wpc * P, C_OUT], dt.float32, kind="ExternalOutput")
    if mode == "bf16_pre":
        feat_bf = nc.dram_tensor("feat_bf", [n_feat_rows, C_IN], dt.bfloat16)

    with tile.TileContext(nc) as tc:
        with (
            tc.tile_pool(name="consts", bufs=1) as cpool,
            tc.tile_pool(name="g", bufs=3) as gpool,
            tc.tile_pool(name="h", bufs=3) as hpool,
            tc.tile_pool(name="aggs", bufs=2) as apool,
            tc.tile_pool(name="outs", bufs=2) as opool,
            tc.tile_pool(name="psa", bufs=2, space="PSUM") as psa,
            tc.tile_pool(name="pso", bufs=2, space="PSUM") as pso,
        ):
            if mode == "bf16_pre":
                # one-time cast of the gather source via SBUF bounce
                # (DRAM->DRAM SWDGE cast crashes the device on this runtime)
                nb = n_feat_rows // P
                rem = n_feat_rows - nb * P
                CCH = 26
                with tc.tile_pool(name="cast", bufs=2) as castpool:
                    fview = feat[: nb * P].rearrange("(p a) c -> p a c", p=P)
                    bview = feat_bf[: nb * P].rearrange("(p a) c -> p a c", p=P)
                    for a in range(0, nb, CCH):
                        e = min(a + CCH, nb)
                        cf = castpool.tile([P, CCH, C_IN], dt.float32, tag="cf")
                        cb = castpool.tile([P, CCH, C_IN], dt.bfloat16, tag="cb")
                        nc.sync.dma_start(out=cf[:, : e - a, :], in_=fview[:, a:e, :])
                        nc.vector.tensor_copy(cb[:, : e - a, :], cf[:, : e - a, :])
                        nc.sync.dma_start(out=bview[:, a:e, :], in_=cb[:, : e - a, :])
                    if rem:
                        tf = castpool.tile([P, C_IN], dt.float32, tag="tf")
                        tb = castpool.tile([P, C_IN], dt.bfloat16, tag="tb")
                        nc.sync.dma_start(out=tf[:rem], in_=feat[nb * P :])
                        nc.vector.tensor_copy(tb[:rem], tf[:rem])
                        nc.sync.dma_start(out=feat_bf[nb * P :], in_=tb[:rem])

            iota_s = cpool.tile([P, P], dt.float32)
            w_s = cpool.tile([P, C_OUT], dt.float32)
            bb_s = cpool.tile([P, C_OUT], dt.float32)
            idx_s = cpool.tile([P, wpc, idxcols], dt.int16)
            dst_s = cpool.tile([P, wpc, nch], dt.float32)
            nc.sync.dma_start(out=iota_s[:], in_=iota_d[:])
            nc.sync.dma_start(out=w_s[:], in_=w_d[:])
            nc.sync.dma_start(out=bb_s[:], in_=bb_d[:])
            nc.sync.dma_start(out=idx_s[:].rearrange("p w c -> p (w c)"), in_=idxs_d[:])
            nc.sync.dma_start(out=dst_s[:].rearrange("p w c -> p (w c)"), in_=dstloc_d[:])

            for w in range(wpc):
                groups = [
                    (a, min(a + GATHER_GROUP, nch)) for a in range(0, nch, GATHER_GROUP)
                ]
                if mode == "bf16_pre":
                    g_s = gpool.tile([P, nch, P], dt.bfloat16)
                    gsrc, gdst = feat_bf, g_s
                else:
                    g32 = gpool.tile([P, nch, P], dt.float32, tag="g32")
                    gsrc, gdst = feat, g32
                for gi, (a, e) in enumerate(groups):
                    n = (e - a) * P
                    nc.gpsimd.dma_gather(
                        out_ap=gdst[:, a:e, :],
                        in_ap=gsrc[:],
                        idxs_ap=idx_s[:, w, a * 8 : e * 8],
                        num_idxs=n, num_idxs_reg=n, elem_size=C_IN,
                        queue_num=(w * len(groups) + gi) % 4,
                    )
                if mode == "bf16_dve":
                    g_s = gpool.tile([P, nch, P], dt.bfloat16, tag="g16")
                    nc.vector.tensor_copy(g_s[:], g32[:])
                elif mode == "f32":
                    g_s = g32

                h_s = hpool.tile([P, nch, P], mm_dt)
                nc.vector.tensor_tensor(
                    out=h_s[:],
                    in0=iota_s[:, None, :].to_broadcast([P, nch, P]),
                    in1=dst_s[:, w, :, None].to_broadcast([P, nch, P]),
                    op=mybir.AluOpType.is_equal,
                )

                aggt_p = psa.tile([P, P], dt.float32)
                for k in range(nch):
                    nc.tensor.matmul(
                        aggt_p[:],
                        lhsT=g_s[:, k, :],
                        rhs=h_s[:, k, :],
                        start=(k == 0),
                        stop=(k == nch - 1),
                    )

                aggt_s = apool.tile([P, P], dt.float32)
                nc.scalar.copy(aggt_s[:], aggt_p[:])

                out_p = pso.tile([P, C_OUT], dt.float32)
                nc.tensor.matmul(out_p[:], lhsT=aggt_s[:], rhs=w_s[:], start=True, stop=True)

                out_t = opool.tile([P, C_OUT], dt.float32)
                nc.vector.tensor_add(out_t[:], out_p[:], bb_s[:])
                nc.sync.dma_start(out=out_d[w * P : (w + 1) * P, :], in_=out_t[:])

    nc.compile()
    return nc


def _prep_inputs(features, edge_index, W, b, n_cores: int, wpc: int):
    """Host-side sharding: group edges by dst window, pad, build per-core maps."""
    nw_total = n_cores * wpc

    src = np.asarray(edge_index[0], dtype=np.int64)
    dst = np.asarray(edge_index[1], dtype=np.int64)
    win = dst // P
    order = np.argsort(win, kind="stable")
    src_s = src[order].astype(np.int16)
    dl_s = (dst[order] % P).astype(np.float32)
    counts = np.bincount(win, minlength=nw_total)
    offs = np.zeros(nw_total + 1, dtype=np.int64)
    np.cumsum(counts, out=offs[1:])

    nch = max(1, int(np.ceil(counts.max() / P)))
    epw = nch * P
    idx_pad = np.zeros((nw_total, epw), dtype=np.int16)
    dl_pad = np.full((nw_total, epw), -1.0, dtype=np.float32)
    for w in range(nw_total):
        cnt = counts[w]
        idx_pad[w, :cnt] = src_s[offs[w] : offs[w + 1]]
        dl_pad[w, :cnt] = dl_s[offs[w] : offs[w + 1]]

    # idxs: value i at [i%16, i//16] -> [16, epw//16] block, replicated to all
    # 8 GPSIMD-core partition groups (each Q7 core reads its own group)
    idxs_all = np.tile(
        idx_pad.reshape(nw_total, epw // 16, 16).transpose(0, 2, 1), (1, 8, 1)
    )
    # dstloc: value i at [i%128, i//128] -> [128, nch]
    dl_all = dl_pad.reshape(nw_total, nch, P).transpose(0, 2, 1)

    feat_np = np.ascontiguousarray(np.asarray(features, dtype=np.float32))
    w_np = np.ascontiguousarray(np.asarray(W, dtype=np.float32))
    bb_np = np.tile(np.asarray(b, dtype=np.float32)[None, :], (P, 1))
    iota_np = np.tile(np.arange(P, dtype=np.float32)[None, :], (P, 1))

    in_maps = []
    for c in range(n_cores):
        sl = slice(c * wpc, (c + 1) * wpc)
        in_maps.append(
            {
                "feat": feat_np,
                "w": w_np,
                "bb": bb_np,
                "iota": iota_np,
                "idxs": np.ascontiguousarray(
                    idxs_all[sl].transpose(1, 0, 2).reshape(P, -1)
                ),
                "dstloc": np.ascontiguousarray(
                    dl_all[sl].transpose(1, 0, 2).reshape(P, -1)
                ),
            }
        )
    return in_maps, nch


_KERNEL_CACHE: dict = {}


def _get_kernel(nch: int):
    key = (N_NODES, WPC, nch, MODE)
    if key not in _KERNEL_CACHE:
        _KERNEL_CACHE[key] = _build_kernel(N_NODES, WPC, nch, MODE)
    return _KERNEL_CACHE[key]


def kernel(features, edge_index, W, b):
    features = np.asarray(features, dtype=np.float32)
    edge_index = np.asarray(edge_index)
    W = np.asarray(W, dtype=np.float32)
    b = np.asarray(b, dtype=np.float32)
    assert features.shape == (N_NODES, C_IN), features.shape
    assert W.shape == (C_IN, C_OUT) and b.shape == (C_OUT,)

    in_maps, nch = _prep_inputs(features, edge_index, W, b, N_CORES, WPC)
    nc = _get_kernel(nch)
    res = bass_utils.run_bass_kernel_spmd(nc, in_maps, core_ids=list(range(N_CORES)))
    out = np.concatenate([res.results[c]["out"] for c in range(N_CORES)], axis=0)
    return np.ascontiguousarray(out[:N_NODES]).astype(np.float32)



# revision 2
# speedup vs baseline: 3.0621x; 3.0621x over previous
"""nn_GCNConv Trainium2 Bass kernel (8 NeuronCores, SPMD, no collectives).

Computation: out = segment_sum(features[src], dst, N) @ W + b
  features [10000,128] f32, edge_index [2,640000] i64, W [128,256], b [256]

Strategy (dense count-matrix SpMM -> pure streaming GEMM, no SWDGE gather):
  - The segment-sum is  agg = A @ F  where A[d,s] = #edges s->d.  The host
    builds A as a dense fp8 count matrix (counts are tiny ints, exact in
    fp8e4) sharded by dst: core c owns dst nodes [1280c, 1280c+1280).
  - Per core the device computes, over 80 src chunks of 128:
      aggT[f,d] += F_chunk[s,f]^T @ A_chunk[s,d]    (PE, bf16 x fp8, PSUM f32)
    with dst split into 3 PSUM groups (512/512/256 cols).  A streams from
    HBM in k-blocks (double-buffered); F streams once and stays resident.
  - Stage 2 per 128-dst window:  out = aggT^T @ W + b  (bf16 matmul + DVE
    bias add), overlapped with the next group's accumulation loop.
  - Host concatenates the 8 per-core [1280,256] outputs, truncates to 10000.
    Only index bookkeeping (histogram/pack) happens on host; all arithmetic
    on feature values runs on device.
"""

import sys

import ml_dtypes
import numpy as np

_TRN_REPO = "/opt/trn_rl_repo"
if _TRN_REPO not in sys.path:
    sys.path.insert(0, _TRN_REPO)

import concourse.bass as bass  # noqa: E402
import concourse.mybir as mybir  # noqa: E402
import concourse.tile as tile  # noqa: E402
from concourse import bacc, bass_utils  # noqa: E402

# ---------------------------------------------------------------------------
# Workaround: this walrus build rejects >1 sync-wait on a CTRL instruction
# ("Too many sync wait commands"). Tile's tail drain attaches a wait for every
# live sem lane to one InstDrain; chunk them across single-wait nops instead.
import re as _re  # noqa: E402

import bass_rust as _bass_rust  # noqa: E402


def _clock_ticks(vc):
    m = _re.search(r"\[([0-9, ]*)\]", repr(vc))
    return [int(x) for x in m.group(1).split(",")] if m.group(1).strip() else []


def _drain_and_barrier(self, tick_clock, wait_clock):
    ticks = _clock_ticks(tick_clock.global_clock)
    nz = [(i, t) for i, t in enumerate(ticks) if t > 0]
    for i, t in nz:
        vc = _bass_rust.VectorClock()
        vc.require_at_least(i, t)
        nop = self.nc.sync.nop(nofuse=True, hint="tail_wait")
        wait_clock.add_sem_waits(nop.ins, tile.ScopedClock({None: vc}))
    self.nc.sync.drain()  # waits already carried by the nops (SP FIFO order)
    self.nc.all_engine_barrier()
    assert self.sems is not None
    popped = self.nc._tile_sem_poison_stack.pop()
    assert popped is self._sem_poison
    self.nc.clear_and_free_semaphores(list(self.sems.allocated().values()))
    self.nc.all_engine_barrier()


tile.TileContext._drain_and_barrier = _drain_and_barrier
# ---------------------------------------------------------------------------

P = 128            # SBUF partitions = window node count = src chunk size
C_IN = 128
C_OUT = 256
N_NODES = 10000
N_CORES = 8
DPC = 1280         # dst nodes per core
NCH = 80           # src chunks (10240 padded src rows / 128)
GROUPS = (512, 512, 256)   # dst columns per PSUM accumulation group
KB = 5             # src chunks per streamed A block


def _build_kernel():
    nc = bacc.Bacc("TRN2")
    dt = mybir.dt

    a_d = [
        nc.dram_tensor(f"a{gi}", [P, NCH, ng], dt.float8e4, kind="ExternalInput")
        for gi, ng in enumerate(GROUPS)
    ]
    f_d = nc.dram_tensor("f", [P, NCH, C_IN], dt.bfloat16, kind="ExternalInput")
    w_d = nc.dram_tensor("w", [C_IN, C_OUT], dt.bfloat16, kind="ExternalInput")
    bb_d = nc.dram_tensor("bb", [P, C_OUT], dt.float32, kind="ExternalInput")
    out_d = nc.dram_tensor("out", [DPC, C_OUT], dt.float32, kind="ExternalOutput")

    with tile.TileContext(nc) as tc:
        with (
            tc.tile_pool(name="consts", bufs=1) as cpool,
            tc.tile_pool(name="a", bufs=3) as apool,
            tc.tile_pool(name="agg", bufs=2) as aggpool,
            tc.tile_pool(name="o", bufs=2) as opool,
            tc.tile_pool(name="psa", bufs=2, space="PSUM") as psa,
            tc.tile_pool(name="pso", bufs=2, space="PSUM") as pso,
        ):
            w_s = cpool.tile([C_IN, C_OUT], dt.bfloat16)
            bb_s = cpool.tile([P, C_OUT], dt.float32)
            f_s = cpool.tile([P, NCH, C_IN], dt.bfloat16)
            nc.sync.dma_start(out=w_s[:], in_=w_d[:])
            nc.sync.dma_start(out=bb_s[:], in_=bb_d[:])

            dbase = 0
            for gi, ng in enumerate(GROUPS):
                aggp = psa.tile([P, ng], dt.float32, tag="aggp", padded_shape=[P, 512])
                for kb in range(0, NCH, KB):
                    a_t = apool.tile([P, KB, ng], dt.float8e4, tag="a",
                                     padded_shape=[P, KB, 512])
                    nc.sync.dma_start(out=a_t[:, :, :ng], in_=a_d[gi][:, kb : kb + KB, :])
                    if gi == 0:
                        nc.sync.dma_start(
                            out=f_s[:, kb : kb + KB, :], in_=f_d[:, kb : kb + KB, :]
                        )
                    for k in range(KB):
                        kk = kb + k
                        nc.tensor.matmul(
                            aggp[:],
                            lhsT=f_s[:, kk, :],
                            rhs=a_t[:, k, :ng],
                            start=(kk == 0),
                            stop=(kk == NCH - 1),
                        )

                agg_s = aggpool.tile([P, ng], dt.bfloat16, tag="agg",
                                     padded_shape=[P, 512])
                nc.vector.tensor_copy(agg_s[:, :ng], aggp[:])

                for wl in range(ng // P):
                    outp = pso.tile([P, C_OUT], dt.float32)
                    nc.tensor.matmul(
                        outp[:],
                        lhsT=agg_s[:, wl * P : (wl + 1) * P],
                        rhs=w_s[:],
                        start=True,
                        stop=True,
                    )
                    out_t = opool.tile([P, C_OUT], dt.float32)
                    nc.vector.tensor_add(out_t[:], outp[:], bb_s[:])
                    row = dbase + wl * P
                    nc.sync.dma_start(out=out_d[row : row + P, :], in_=out_t[:])
                dbase += ng

    nc.compile()
    return nc


def _prep_inputs(features, edge_index, W, b):
    """Host-side sharding: dense per-core fp8 count matrices + packed F/W/b."""
    src = np.asarray(edge_index[0]).astype(np.int64)
    dst = np.asarray(edge_index[1]).astype(np.int64)

    # A[core, p, c, dloc] = #edges (src = c*128+p) -> (dst = core*1280+dloc)
    A = np.zeros((N_CORES, P, NCH, DPC), np.uint8)
    flat = ((dst // DPC * P + src % P) * NCH + src // P) * DPC + dst % DPC
    np.add.at(A.reshape(-1), flat, 1)
    amax = int(A.max())
    assert amax <= 16, f"edge multiplicity {amax} not exact in fp8"
    A8 = A.astype(ml_dtypes.float8_e4m3)

    bounds = np.cumsum((0,) + GROUPS)
    a_groups = [
        np.ascontiguousarray(A8[:, :, :, bounds[gi] : bounds[gi + 1]])
        for gi in range(len(GROUPS))
    ]

    f16 = np.zeros((NCH * P, C_IN), ml_dtypes.bfloat16)
    f16[:N_NODES] = np.asarray(features, np.float32).astype(ml_dtypes.bfloat16)
    f_host = np.ascontiguousarray(f16.reshape(NCH, P, C_IN).transpose(1, 0, 2))
    w_host = np.asarray(W, np.float32).astype(ml_dtypes.bfloat16)
    bb_host = np.tile(np.asarray(b, np.float32)[None, :], (P, 1))

    in_maps = []
    for ci in range(N_CORES):
        m = {f"a{gi}": a_groups[gi][ci] for gi in range(len(GROUPS))}
        m.update({"f": f_host, "w": w_host, "bb": bb_host})
        in_maps.append(m)
    return in_maps


_KERNEL_CACHE: dict = {}


def _get_kernel():
    if "k" not in _KERNEL_CACHE:
        _KERNEL_CACHE["k"] = _build_kernel()
    return _KERNEL_CACHE["k"]


def kernel(features, edge_index, W, b):
    features = np.asarray(features, dtype=np.float32)
    edge_index = np.asarray(edge_index)
    W = np.asarray(W, dtype=np.float32)
    b = np.asarray(b, dtype=np.float32)
    assert features.shape == (N_NODES, C_IN), features.shape
    assert W.shape == (C_IN, C_OUT) and b.shape == (C_OUT,)

    in_maps = _prep_inputs(features, edge_index, W, b)
    nc = _get_kernel()
    res = bass_utils.run_bass_kernel_spmd(nc, in_maps, core_ids=list(range(N_CORES)))
    out = np.concatenate([res.results[c]["out"] for c in range(N_CORES)], axis=0)
    return np.ascontiguousarray(out[:N_NODES]).astype(np.float32)


# revision 4
# speedup vs baseline: 3.3155x; 1.0828x over previous
"""nn_GCNConv Trainium2 Bass kernel (8 NeuronCores, SPMD, no collectives).

Computation: out = segment_sum(features[src], dst, N) @ W + b
  features [10000,128] f32, edge_index [2,640000] i64, W [128,256], b [256]

Strategy (dense count-matrix SpMM -> pure streaming GEMM, no SWDGE gather):
  - The segment-sum is  agg = A @ F  where A[d,s] = #edges s->d.  The host
    builds A as a dense fp8 count matrix (counts are tiny ints, exact in
    fp8e4) sharded by dst: core c owns dst nodes [1280c, 1280c+1280).
  - Per core the device computes, over 80 src chunks of 128:
      aggT[f,d] += F_chunk[s,f]^T @ A_chunk[s,d]    (PE, bf16 x fp8, PSUM f32)
    with dst split into 3 PSUM groups (512/512/256 cols).  A streams from
    HBM in 4 blocks per group (double-buffered, issued alternately on the
    two HWDGE rings SP/Activation); F streams once and stays resident.
  - Stage 2 per 128-dst window:  out = aggT^T @ W + b  (bf16 matmul + DVE
    bias add); its PE work is delayed one group so the PE never waits on
    the PSUM->SBUF copy.  Output is written p-major ([128,10,256]) so each
    group's store is one big-line DMA; the host untransposes.
  - Host concatenates the 8 per-core outputs and truncates to 10000 rows.
    Only index bookkeeping (histogram/pack) happens on host; all arithmetic
    on feature values runs on device.
"""

import sys

import ml_dtypes
import numpy as np

_TRN_REPO = "/opt/trn_rl_repo"
if _TRN_REPO not in sys.path:
    sys.path.insert(0, _TRN_REPO)

import concourse.bass as bass  # noqa: E402
import concourse.mybir as mybir  # noqa: E402
import concourse.tile as tile  # noqa: E402
from concourse import bacc, bass_utils  # noqa: E402

# ---------------------------------------------------------------------------
# Workaround: this walrus build rejects >1 sync-wait on a CTRL instruction
# ("Too many sync wait commands"). Tile's tail drain attaches a wait for every
# live sem lane to one InstDrain; chunk them across single-wait nops instead.
import re as _re  # noqa: E402

import bass_rust as _bass_rust  # noqa: E402


def _clock_ticks(vc):
    m = _re.search(r"\[([0-9, ]*)\]", repr(vc))
    return [int(x) for x in m.group(1).split(",")] if m.group(1).strip() else []


def _drain_and_barrier(self, tick_clock, wait_clock):
    ticks = _clock_ticks(tick_clock.global_clock)
    nz = [(i, t) for i, t in enumerate(ticks) if t > 0]
    for i, t in nz:
        vc = _bass_rust.VectorClock()
        vc.require_at_least(i, t)
        nop = self.nc.sync.nop(nofuse=True, hint="tail_wait")
        wait_clock.add_sem_waits(nop.ins, tile.ScopedClock({None: vc}))
    self.nc.sync.drain()  # waits already carried by the nops (SP FIFO order)
    self.nc.all_engine_barrier()
    assert self.sems is not None
    popped = self.nc._tile_sem_poison_stack.pop()
    assert popped is self._sem_poison
    self.nc.clear_and_free_semaphores(list(self.sems.allocated().values()))
    self.nc.all_engine_barrier()


tile.TileContext._drain_and_barrier = _drain_and_barrier
# ---------------------------------------------------------------------------

P = 128            # SBUF partitions = window node count = src chunk size
C_IN = 128
C_OUT = 256
N_NODES = 10000
N_CORES = 8
DPC = 1280         # dst nodes per core
NWIN = DPC // P    # 10 dst windows per core
NCH = 80           # src chunks (10240 padded src rows / 128)
GROUPS = (512, 512, 256)   # dst columns per PSUM accumulation group
KB = 20            # src chunks per streamed A block


def _build_kernel():
    nc = bacc.Bacc("TRN2")
    dt = mybir.dt

    a_d = [
        nc.dram_tensor(f"a{gi}", [P, NCH, ng], dt.float8e4, kind="ExternalInput")
        for gi, ng in enumerate(GROUPS)
    ]
    f_d = nc.dram_tensor("f", [P, NCH, C_IN], dt.bfloat16, kind="ExternalInput")
    w_d = nc.dram_tensor("w", [C_IN, C_OUT], dt.bfloat16, kind="ExternalInput")
    bb_d = nc.dram_tensor("bb", [P, C_OUT], dt.float32, kind="ExternalInput")
    out_d = nc.dram_tensor("out", [P, NWIN, C_OUT], dt.float32, kind="ExternalOutput")

    dmae = [nc.sync, nc.scalar]   # the two HWDGE rings; alternate issues

    with tile.TileContext(nc) as tc:
        with (
            tc.tile_pool(name="consts", bufs=1) as cpool,
            tc.tile_pool(name="a", bufs=2) as apool,
            tc.tile_pool(name="agg", bufs=2) as aggpool,
            tc.tile_pool(name="o", bufs=2) as opool,
            tc.tile_pool(name="psa", bufs=2, space="PSUM") as psa,
            tc.tile_pool(name="pso", bufs=2, space="PSUM") as pso,
        ):
            w_s = cpool.tile([C_IN, C_OUT], dt.bfloat16)
            bb_s = cpool.tile([P, C_OUT], dt.float32)
            f_s = cpool.tile([P, NCH, C_IN], dt.bfloat16)
            nc.sync.dma_start(out=w_s[:], in_=w_d[:])
            nc.scalar.dma_start(out=bb_s[:], in_=bb_d[:])

            def stage2(gi, agg_s, wbase, ng):
                nw = ng // P
                out_t = opool.tile([P, nw, C_OUT], dt.float32, tag="o",
                                   name=f"out_g{gi}")
                for wl in range(nw):
                    outp = pso.tile([P, C_OUT], dt.float32, tag="op",
                                    name=f"op{gi}_{wl}")
                    nc.tensor.matmul(
                        outp[:],
                        lhsT=agg_s[:, wl * P : (wl + 1) * P],
                        rhs=w_s[:],
                        start=True,
                        stop=True,
                    )
                    nc.vector.tensor_add(out_t[:, wl, :], outp[:], bb_s[:])
                dmae[gi % 2].dma_start(
                    out=out_d[:, wbase : wbase + nw, :], in_=out_t[:]
                )

            pending = None
            wbase = 0
            qi = 0
            for gi, ng in enumerate(GROUPS):
                aggp = psa.tile([P, ng], dt.float32, tag="aggp",
                                padded_shape=[P, 512], name=f"aggp{gi}")
                for kb in range(0, NCH, KB):
                    a_t = apool.tile([P, KB, ng], dt.float8e4, tag=f"a{ng}",
                                     name=f"a_t{gi}_{kb}")
                    dmae[qi % 2].dma_start(
                        out=a_t[:], in_=a_d[gi][:, kb : kb + KB, :]
                    )
                    qi += 1
                    if gi == 0:
                        dmae[qi % 2].dma_start(
                            out=f_s[:, kb : kb + KB, :], in_=f_d[:, kb : kb + KB, :]
                        )
                        qi += 1
                    for k in range(KB):
                        kk = kb + k
                        nc.tensor.matmul(
                            aggp[:],
                            lhsT=f_s[:, kk, :],
                            rhs=a_t[:, k, :],
                            start=(kk == 0),
                            stop=(kk == NCH - 1),
                        )

                agg_s = aggpool.tile([P, ng], dt.bfloat16, tag="agg",
                                     padded_shape=[P, 512], name=f"agg_s{gi}")
                nc.vector.tensor_copy(agg_s[:], aggp[:])

                if pending is not None:
                    stage2(*pending)
                pending = (gi, agg_s, wbase, ng)
                wbase += ng // P
            stage2(*pending)

    nc.compile()
    return nc


def _prep_inputs(features, edge_index, W, b):
    """Host-side sharding: dense per-core fp8 count matrices + packed F/W/b."""
    src = np.asarray(edge_index[0]).astype(np.int64)
    dst = np.asarray(edge_index[1]).astype(np.int64)

    # A[core, p, c, dloc] = #edges (src = c*128+p) -> (dst = core*1280+dloc)
    A = np.zeros((N_CORES, P, NCH, DPC), np.uint8)
    flat = ((dst // DPC * P + src % P) * NCH + src // P) * DPC + dst % DPC
    np.add.at(A.reshape(-1), flat, 1)
    amax = int(A.max())
    assert amax <= 16, f"edge multiplicity {amax} not exact in fp8"
    A8 = A.astype(ml_dtypes.float8_e4m3)

    bounds = np.cumsum((0,) + GROUPS)
    a_groups = [
        np.ascontiguousarray(A8[:, :, :, bounds[gi] : bounds[gi + 1]])
        for gi in range(len(GROUPS))
    ]

    f16 = np.zeros((NCH * P, C_IN), ml_dtypes.bfloat16)
    f16[:N_NODES] = np.asarray(features, np.float32).astype(ml_dtypes.bfloat16)
    f_host = np.ascontiguousarray(f16.reshape(NCH, P, C_IN).transpose(1, 0, 2))
    w_host = np.asarray(W, np.float32).astype(ml_dtypes.bfloat16)
    bb_host = np.tile(np.asarray(b, np.float32)[None, :], (P, 1))

    in_maps = []
    for ci in range(N_CORES):
        m = {f"a{gi}": a_groups[gi][ci] for gi in range(len(GROUPS))}
        m.update({"f": f_host, "w": w_host, "bb": bb_host})
        in_maps.append(m)
    return in_maps


_KERNEL_CACHE: dict = {}


def _get_kernel():
    if "k" not in _KERNEL_CACHE:
        _KERNEL_CACHE["k"] = _build_kernel()
    return _KERNEL_CACHE["k"]


def kernel(features, edge_index, W, b):
    features = np.asarray(features, dtype=np.float32)
    edge_index = np.asarray(edge_index)
    W = np.asarray(W, dtype=np.float32)
    b = np.asarray(b, dtype=np.float32)
    assert features.shape == (N_NODES, C_IN), features.shape
    assert W.shape == (C_IN, C_OUT) and b.shape == (C_OUT,)

    in_maps = _prep_inputs(features, edge_index, W, b)
    nc = _get_kernel()
    res = bass_utils.run_bass_kernel_spmd(nc, in_maps, core_ids=list(range(N_CORES)))
    # out is [128, 10, 256] p-major per core -> [1280, 256] node-major
    out = np.concatenate(
        [
            res.results[c]["out"].transpose(1, 0, 2).reshape(DPC, C_OUT)
            for c in range(N_CORES)
        ],
        axis=0,
    )
    return np.ascontiguousarray(out[:N_NODES]).astype(np.float32)


# revision 7
# speedup vs baseline: 4.2115x; 1.2702x over previous
"""nn_GCNConv Trainium2 Bass kernel (8 NeuronCores, SPMD, no collectives).

Computation: out = segment_sum(features[src], dst, N) @ W + b
  features [10000,128] f32, edge_index [2,640000] i64, W [128,256], b [256]

Strategy (dense count-matrix SpMM -> pure streaming GEMM, no SWDGE gather):
  - The segment-sum is  agg = A @ F  where A[d,s] = #edges s->d.  The host
    builds A as a dense fp8 count matrix (counts are tiny ints, exact in
    fp8e4) sharded by dst: core c owns dst nodes [1280c, 1280c+1280).
  - Per core the device computes, over 80 src chunks of 128:
      aggT[f,d] += F_chunk[s,f]^T @ A_chunk[s,d]    (PE, bf16 x fp8, PSUM f32)
    with dst split into 3 PSUM groups (512/512/256 cols).  A streams from
    HBM in 4 blocks per group (double-buffered, issued alternately on the
    two HWDGE rings SP/Activation); F streams once and stays resident.
  - Stage 2 per 128-dst window:  out = aggT^T @ W + b  (bf16 matmul + DVE
    bias add); its PE work is delayed one group so the PE never waits on
    the PSUM->SBUF copy.  Output is written p-major ([128,10,256]) so each
    group's store is one big-line DMA; the host untransposes.
  - Host concatenates the 8 per-core outputs and truncates to 10000 rows.
    Only index bookkeeping (histogram/pack) happens on host; all arithmetic
    on feature values runs on device.
"""

import sys

import ml_dtypes
import numpy as np

_TRN_REPO = "/opt/trn_rl_repo"
if _TRN_REPO not in sys.path:
    sys.path.insert(0, _TRN_REPO)

import concourse.bass as bass  # noqa: E402
import concourse.mybir as mybir  # noqa: E402
import concourse.tile as tile  # noqa: E402
from concourse import bacc, bass_utils  # noqa: E402

# ---------------------------------------------------------------------------
# Workaround: this walrus build rejects >1 sync-wait on a CTRL instruction
# ("Too many sync wait commands"). Tile's tail drain attaches a wait for every
# live sem lane to one InstDrain; chunk them across single-wait nops instead.
import re as _re  # noqa: E402

import bass_rust as _bass_rust  # noqa: E402


def _clock_ticks(vc):
    m = _re.search(r"\[([0-9, ]*)\]", repr(vc))
    return [int(x) for x in m.group(1).split(",")] if m.group(1).strip() else []


def _drain_and_barrier(self, tick_clock, wait_clock):
    ticks = _clock_ticks(tick_clock.global_clock)
    nz = [(i, t) for i, t in enumerate(ticks) if t > 0]
    for i, t in nz:
        vc = _bass_rust.VectorClock()
        vc.require_at_least(i, t)
        nop = self.nc.sync.nop(nofuse=True, hint="tail_wait")
        wait_clock.add_sem_waits(nop.ins, tile.ScopedClock({None: vc}))
    self.nc.sync.drain()  # waits already carried by the nops (SP FIFO order)
    self.nc.all_engine_barrier()
    assert self.sems is not None
    popped = self.nc._tile_sem_poison_stack.pop()
    assert popped is self._sem_poison
    self.nc.clear_and_free_semaphores(list(self.sems.allocated().values()))
    self.nc.all_engine_barrier()


tile.TileContext._drain_and_barrier = _drain_and_barrier
# ---------------------------------------------------------------------------

P = 128            # SBUF partitions = window node count = src chunk size
C_IN = 128
C_OUT = 256
N_NODES = 10000
N_CORES = 8
DPC = 1280         # dst nodes per core
NWIN = DPC // P    # 10 dst windows per core
NCH = 79           # src chunks (10112 padded src rows / 128)
GROUPS = (512, 512, 256)   # dst columns per PSUM accumulation group
ROUNDS = (4, 8, 17, 17, 17, 16)   # src chunks per streamed round (sums to NCH)


def _build_kernel():
    nc = bacc.Bacc("TRN2")
    dt = mybir.dt

    a_d = [
        nc.dram_tensor(f"a{gi}", [P, NCH, ng], dt.float8e4, kind="ExternalInput")
        for gi, ng in enumerate(GROUPS)
    ]
    f_d = nc.dram_tensor("f", [P, NCH, C_IN], dt.bfloat16, kind="ExternalInput")
    w_d = nc.dram_tensor("w", [C_IN, C_OUT], dt.bfloat16, kind="ExternalInput")
    bb_d = nc.dram_tensor("bb", [P, C_OUT], dt.float32, kind="ExternalInput")
    out_d = nc.dram_tensor("out", [P, NWIN, C_OUT], dt.float16, kind="ExternalOutput")

    dmae = [nc.sync, nc.scalar]   # the two HWDGE rings; alternate issues

    with tile.TileContext(nc) as tc:
        with (
            tc.tile_pool(name="consts", bufs=1) as cpool,
            tc.tile_pool(name="a", bufs=3) as apool,
            tc.tile_pool(name="agg", bufs=1) as aggpool,
            tc.tile_pool(name="o", bufs=2) as opool,
            tc.tile_pool(name="psa", bufs=3, space="PSUM") as psa,
            tc.tile_pool(name="pso", bufs=2, space="PSUM") as pso,
        ):
            w_s = cpool.tile([C_IN, C_OUT], dt.bfloat16)
            bb_s = cpool.tile([P, C_OUT], dt.float32)
            f_s = cpool.tile([P, NCH, C_IN], dt.bfloat16)
            nc.sync.dma_start(out=w_s[:], in_=w_d[:])
            nc.scalar.dma_start(out=bb_s[:], in_=bb_d[:])

            # three persistent PSUM accumulation groups, one per dst slice
            aggps = [
                psa.tile([P, ng], dt.float32, tag="aggp", padded_shape=[P, 512],
                         name=f"aggp{gi}")
                for gi, ng in enumerate(GROUPS)
            ]

            qi = 0
            kb0 = 0
            for ri, kbn in enumerate(ROUNDS):
                dmae[qi % 2].dma_start(
                    out=f_s[:, kb0 : kb0 + kbn, :], in_=f_d[:, kb0 : kb0 + kbn, :]
                )
                qi += 1
                for gi, ng in enumerate(GROUPS):
                    a_t = apool.tile([P, kbn, ng], dt.float8e4, tag=f"a{ng}",
                                     padded_shape=[P, max(ROUNDS), ng],
                                     name=f"a_t{gi}_{ri}")
                    dmae[qi % 2].dma_start(
                        out=a_t[:, :kbn, :], in_=a_d[gi][:, kb0 : kb0 + kbn, :]
                    )
                    qi += 1
                    for k in range(kbn):
                        kk = kb0 + k
                        nc.tensor.matmul(
                            aggps[gi][:],
                            lhsT=f_s[:, kk, :],
                            rhs=a_t[:, k, :],
                            start=(kk == 0),
                            stop=(kk == NCH - 1),
                        )
                kb0 += kbn

            # stage 2: copies drain on DVE while the PE finishes the last
            # round; s2 matmuls are emitted after all accumulation matmuls
            agg_ss = []
            for gi, ng in enumerate(GROUPS):
                agg_s = aggpool.tile([P, ng], dt.bfloat16, tag=f"agg{gi}",
                                     name=f"agg_s{gi}")
                nc.vector.tensor_copy(agg_s[:], aggps[gi][:])
                agg_ss.append(agg_s)

            wbase = 0
            for gi, ng in enumerate(GROUPS):
                nw = ng // P
                out_t = opool.tile([P, nw, C_OUT], dt.float16, tag="o",
                                   name=f"out_g{gi}")
                for wl in range(nw):
                    outp = pso.tile([P, C_OUT], dt.float32, tag="op",
                                    name=f"op{gi}_{wl}")
                    nc.tensor.matmul(
                        outp[:],
                        lhsT=agg_ss[gi][:, wl * P : (wl + 1) * P],
                        rhs=w_s[:],
                        start=True,
                        stop=True,
                    )
                    nc.vector.tensor_add(out_t[:, wl, :], outp[:], bb_s[:])
                dmae[gi % 2].dma_start(
                    out=out_d[:, wbase : wbase + nw, :], in_=out_t[:]
                )
                wbase += nw

    nc.compile()
    return nc


def _prep_inputs(features, edge_index, W, b):
    """Host-side sharding: dense per-core fp8 count matrices + packed F/W/b."""
    src = np.asarray(edge_index[0]).astype(np.int64)
    dst = np.asarray(edge_index[1]).astype(np.int64)

    # A[core, p, c, dloc] = #edges (src = c*128+p) -> (dst = core*1280+dloc)
    A = np.zeros((N_CORES, P, NCH, DPC), np.uint8)
    flat = ((dst // DPC * P + src % P) * NCH + src // P) * DPC + dst % DPC
    np.add.at(A.reshape(-1), flat, 1)
    amax = int(A.max())
    assert amax <= 16, f"edge multiplicity {amax} not exact in fp8"
    A8 = A.astype(ml_dtypes.float8_e4m3)

    bounds = np.cumsum((0,) + GROUPS)
    a_groups = [
        np.ascontiguousarray(A8[:, :, :, bounds[gi] : bounds[gi + 1]])
        for gi in range(len(GROUPS))
    ]

    f16 = np.zeros((NCH * P, C_IN), ml_dtypes.bfloat16)
    f16[:N_NODES] = np.asarray(features, np.float32).astype(ml_dtypes.bfloat16)
    f_host = np.ascontiguousarray(f16.reshape(NCH, P, C_IN).transpose(1, 0, 2))
    w_host = np.asarray(W, np.float32).astype(ml_dtypes.bfloat16)
    bb_host = np.tile(np.asarray(b, np.float32)[None, :], (P, 1))

    in_maps = []
    for ci in range(N_CORES):
        m = {f"a{gi}": a_groups[gi][ci] for gi in range(len(GROUPS))}
        m.update({"f": f_host, "w": w_host, "bb": bb_host})
        in_maps.append(m)
    return in_maps


_KERNEL_CACHE: dict = {}


def _get_kernel():
    if "k" not in _KERNEL_CACHE:
        _KERNEL_CACHE["k"] = _build_kernel()
    return _KERNEL_CACHE["k"]


def kernel(features, edge_index, W, b):
    features = np.asarray(features, dtype=np.float32)
    edge_index = np.asarray(edge_index)
    W = np.asarray(W, dtype=np.float32)
    b = np.asarray(b, dtype=np.float32)
    assert features.shape == (N_NODES, C_IN), features.shape
    assert W.shape == (C_IN, C_OUT) and b.shape == (C_OUT,)

    in_maps = _prep_inputs(features, edge_index, W, b)
    nc = _get_kernel()
    res = bass_utils.run_bass_kernel_spmd(nc, in_maps, core_ids=list(range(N_CORES)))
    # out is [128, 10, 256] f16 p-major per core -> [1280, 256] node-major
    out = np.concatenate(
        [
            np.asarray(res.results[c]["out"], np.float32)
            .transpose(1, 0, 2)
            .reshape(DPC, C_OUT)
            for c in range(N_CORES)
        ],
        axis=0,
    )
    return np.ascontiguousarray(out[:N_NODES])


# revision 9
# speedup vs baseline: 4.2472x; 1.0085x over previous
"""nn_GCNConv Trainium2 Bass kernel (8 NeuronCores, SPMD, no collectives).

Computation: out = segment_sum(features[src], dst, N) @ W + b
  features [10000,128] f32, edge_index [2,640000] i64, W [128,256], b [256]

Strategy (dense count-matrix SpMM -> pure streaming GEMM, no SWDGE gather):
  - The segment-sum is  agg = A @ F  where A[d,s] = #edges s->d.  The host
    builds A as a dense fp8 count matrix (counts are tiny ints, exact in
    fp8e4) sharded by dst: core c owns dst nodes [1280c, 1280c+1280).
  - Per core the device computes, over 80 src chunks of 128:
      aggT[f,d] += F_chunk[s,f]^T @ A_chunk[s,d]    (PE, bf16 x fp8, PSUM f32)
    with dst split into 3 PSUM groups (512/512/256 cols).  A streams from
    HBM in 4 blocks per group (double-buffered, issued alternately on the
    two HWDGE rings SP/Activation); F streams once and stays resident.
  - Stage 2 per 128-dst window:  out = aggT^T @ W + b  (bf16 matmul + DVE
    bias add); its PE work is delayed one group so the PE never waits on
    the PSUM->SBUF copy.  Output is written p-major ([128,10,256]) so each
    group's store is one big-line DMA; the host untransposes.
  - Host concatenates the 8 per-core outputs and truncates to 10000 rows.
    Only index bookkeeping (histogram/pack) happens on host; all arithmetic
    on feature values runs on device.
"""

import sys

import ml_dtypes
import numpy as np

_TRN_REPO = "/opt/trn_rl_repo"
if _TRN_REPO not in sys.path:
    sys.path.insert(0, _TRN_REPO)

import concourse.bass as bass  # noqa: E402
import concourse.mybir as mybir  # noqa: E402
import concourse.tile as tile  # noqa: E402
from concourse import bacc, bass_utils  # noqa: E402

# ---------------------------------------------------------------------------
# Workaround: this walrus build rejects >1 sync-wait on a CTRL instruction
# ("Too many sync wait commands"). Tile's tail drain attaches a wait for every
# live sem lane to one InstDrain; chunk them across single-wait nops instead.
import re as _re  # noqa: E402

import bass_rust as _bass_rust  # noqa: E402


def _clock_ticks(vc):
    m = _re.search(r"\[([0-9, ]*)\]", repr(vc))
    return [int(x) for x in m.group(1).split(",")] if m.group(1).strip() else []


def _drain_and_barrier(self, tick_clock, wait_clock):
    ticks = _clock_ticks(tick_clock.global_clock)
    nz = [(i, t) for i, t in enumerate(ticks) if t > 0]
    for i, t in nz:
        vc = _bass_rust.VectorClock()
        vc.require_at_least(i, t)
        nop = self.nc.sync.nop(nofuse=True, hint="tail_wait")
        wait_clock.add_sem_waits(nop.ins, tile.ScopedClock({None: vc}))
    self.nc.sync.drain()  # waits already carried by the nops (SP FIFO order)
    self.nc.all_engine_barrier()
    assert self.sems is not None
    popped = self.nc._tile_sem_poison_stack.pop()
    assert popped is self._sem_poison
    self.nc.clear_and_free_semaphores(list(self.sems.allocated().values()))
    self.nc.all_engine_barrier()


tile.TileContext._drain_and_barrier = _drain_and_barrier
# ---------------------------------------------------------------------------

P = 128            # SBUF partitions = window node count = src chunk size
C_IN = 128
C_OUT = 256
N_NODES = 10000
N_CORES = 8
DPC = 1280         # dst nodes per core
NWIN = DPC // P    # 10 dst windows per core
NCH = 79           # src chunks (10112 padded src rows / 128)
GROUPS = (512, 512, 256)   # dst columns per PSUM accumulation group
ROUNDS = (4, 8, 17, 17, 17, 16)   # src chunks per streamed round (sums to NCH)


def _build_kernel():
    nc = bacc.Bacc("TRN2")
    dt = mybir.dt

    a_d = [
        nc.dram_tensor(f"a{gi}", [P, NCH, ng], dt.float8e4, kind="ExternalInput")
        for gi, ng in enumerate(GROUPS)
    ]
    f_d = nc.dram_tensor("f", [P, NCH, C_IN], dt.bfloat16, kind="ExternalInput")
    w_d = nc.dram_tensor("w", [C_IN, C_OUT], dt.bfloat16, kind="ExternalInput")
    bb_d = nc.dram_tensor("bb", [P, C_OUT], dt.float32, kind="ExternalInput")
    out_d = nc.dram_tensor("out", [P, NWIN, C_OUT], dt.float16, kind="ExternalOutput")

    with tile.TileContext(nc) as tc:
        with (
            tc.tile_pool(name="consts", bufs=1) as cpool,
            tc.tile_pool(name="a", bufs=3) as apool,
            tc.tile_pool(name="agg", bufs=1) as aggpool,
            tc.tile_pool(name="o", bufs=2) as opool,
            tc.tile_pool(name="psa", bufs=3, space="PSUM") as psa,
            tc.tile_pool(name="pso", bufs=2, space="PSUM") as pso,
        ):
            # HAM pre-warm: a few dummy matmuls on zeroed scratch keep the PE
            # busy during the DMA head so the real stream starts at 2.4 GHz.
            warm_w = cpool.tile([P, C_IN], dt.bfloat16)
            warm_x = cpool.tile([P, 512], dt.float8e4)
            warm_p = psa.tile([P, 512], dt.float32, tag="warm", bufs=1)
            nc.gpsimd.memset(warm_w[:], 0.0)
            nc.gpsimd.memset(warm_x[:], 0.0)
            for _ in range(5):
                nc.tensor.matmul(warm_p[:], lhsT=warm_w[:], rhs=warm_x[:],
                                 start=True, stop=True)

            w_s = cpool.tile([C_IN, C_OUT], dt.bfloat16)
            bb_s = cpool.tile([P, C_OUT], dt.float32)
            f_s = cpool.tile([P, NCH, C_IN], dt.bfloat16)
            nc.gpsimd.dma_start(out=w_s[:], in_=w_d[:])
            nc.gpsimd.dma_start(out=bb_s[:], in_=bb_d[:])

            # three persistent PSUM accumulation groups, one per dst slice
            aggps = [
                psa.tile([P, ng], dt.float32, tag="aggp", padded_shape=[P, 512],
                         name=f"aggp{gi}")
                for gi, ng in enumerate(GROUPS)
            ]

            # dedicated issue streams, ordered to match per-round consumption:
            # scalar ring carries F then a1; sync ring carries a0 then a2
            a_eng = [nc.sync, nc.scalar, nc.sync]
            kb0 = 0
            for ri, kbn in enumerate(ROUNDS):
                nc.scalar.dma_start(
                    out=f_s[:, kb0 : kb0 + kbn, :], in_=f_d[:, kb0 : kb0 + kbn, :]
                )
                for gi, ng in enumerate(GROUPS):
                    a_t = apool.tile([P, kbn, ng], dt.float8e4, tag=f"g{gi}",
                                     padded_shape=[P, max(ROUNDS), ng],
                                     name=f"a_t{gi}_{ri}")
                    a_eng[gi].dma_start(
                        out=a_t[:, :kbn, :], in_=a_d[gi][:, kb0 : kb0 + kbn, :]
                    )
                    for k in range(kbn):
                        kk = kb0 + k
                        nc.tensor.matmul(
                            aggps[gi][:],
                            lhsT=f_s[:, kk, :],
                            rhs=a_t[:, k, :],
                            start=(kk == 0),
                            stop=(kk == NCH - 1),
                        )
                kb0 += kbn

            # stage 2: copies drain on DVE while the PE finishes the last
            # round; s2 matmuls are emitted after all accumulation matmuls
            agg_ss = []
            for gi, ng in enumerate(GROUPS):
                agg_s = aggpool.tile([P, ng], dt.bfloat16, tag=f"agg{gi}",
                                     name=f"agg_s{gi}")
                nc.vector.tensor_copy(agg_s[:], aggps[gi][:])
                agg_ss.append(agg_s)

            wbase = 0
            for gi, ng in enumerate(GROUPS):
                nw = ng // P
                out_t = opool.tile([P, nw, C_OUT], dt.float16, tag="o",
                                   name=f"out_g{gi}")
                for wl in range(nw):
                    outp = pso.tile([P, C_OUT], dt.float32, tag="op",
                                    name=f"op{gi}_{wl}")
                    nc.tensor.matmul(
                        outp[:],
                        lhsT=agg_ss[gi][:, wl * P : (wl + 1) * P],
                        rhs=w_s[:],
                        start=True,
                        stop=True,
                    )
                    nc.vector.tensor_add(out_t[:, wl, :], outp[:], bb_s[:])
                (nc.scalar if gi % 2 else nc.sync).dma_start(
                    out=out_d[:, wbase : wbase + nw, :], in_=out_t[:]
                )
                wbase += nw

    nc.compile()
    return nc


def _prep_inputs(features, edge_index, W, b):
    """Host-side sharding: dense per-core fp8 count matrices + packed F/W/b."""
    src = np.asarray(edge_index[0]).astype(np.int64)
    dst = np.asarray(edge_index[1]).astype(np.int64)

    # A[core, p, c, dloc] = #edges (src = c*128+p) -> (dst = core*1280+dloc)
    A = np.zeros((N_CORES, P, NCH, DPC), np.uint8)
    flat = ((dst // DPC * P + src % P) * NCH + src // P) * DPC + dst % DPC
    np.add.at(A.reshape(-1), flat, 1)
    amax = int(A.max())
    assert amax <= 16, f"edge multiplicity {amax} not exact in fp8"
    A8 = A.astype(ml_dtypes.float8_e4m3)

    bounds = np.cumsum((0,) + GROUPS)
    a_groups = [
        np.ascontiguousarray(A8[:, :, :, bounds[gi] : bounds[gi + 1]])
        for gi in range(len(GROUPS))
    ]

    f16 = np.zeros((NCH * P, C_IN), ml_dtypes.bfloat16)
    f16[:N_NODES] = np.asarray(features, np.float32).astype(ml_dtypes.bfloat16)
    f_host = np.ascontiguousarray(f16.reshape(NCH, P, C_IN).transpose(1, 0, 2))
    w_host = np.asarray(W, np.float32).astype(ml_dtypes.bfloat16)
    bb_host = np.tile(np.asarray(b, np.float32)[None, :], (P, 1))

    in_maps = []
    for ci in range(N_CORES):
        m = {f"a{gi}": a_groups[gi][ci] for gi in range(len(GROUPS))}
        m.update({"f": f_host, "w": w_host, "bb": bb_host})
        in_maps.append(m)
    return in_maps


_KERNEL_CACHE: dict = {}


def _get_kernel():
    if "k" not in _KERNEL_CACHE:
        _KERNEL_CACHE["k"] = _build_kernel()
    return _KERNEL_CACHE["k"]


def kernel(features, edge_index, W, b):
    features = np.asarray(features, dtype=np.float32)
    edge_index = np.asarray(edge_index)
    W = np.asarray(W, dtype=np.float32)
    b = np.asarray(b, dtype=np.float32)
    assert features.shape == (N_NODES, C_IN), features.shape
    assert W.shape == (C_IN, C_OUT) and b.shape == (C_OUT,)

    in_maps = _prep_inputs(features, edge_index, W, b)
    nc = _get_kernel()
    res = bass_utils.run_bass_kernel_spmd(nc, in_maps, core_ids=list(range(N_CORES)))
    # out is [128, 10, 256] f16 p-major per core -> [1280, 256] node-major
    out = np.concatenate(
        [
            np.asarray(res.results[c]["out"], np.float32)
            .transpose(1, 0, 2)
            .reshape(DPC, C_OUT)
            for c in range(N_CORES)
        ],
        axis=0,
    )
    return np.ascontiguousarray(out[:N_NODES])
